# revision 5
# baseline (speedup 1.0000x reference)
"""Segment-mean (MeanAggregator) Trainium2 kernel.

Problem: atom_hiddens [2_000_000, 128] f32, segment_ids = repeat(arange(100_000), 20)
(uniform 20 atoms per molecule), output = per-molecule mean [100_000, 128] f32.

Strategy (8 NeuronCores, data-parallel over molecules):
  - Each core handles 12_500 molecules = 250_000 contiguous atom rows (128 MB).
  - Segment-sum is a matmul against constant 0/1 selection matrices:
      out[m, h] = sum_a S[a, m] * X[a, h]
    For a 2560-atom group (=128 molecules), split atoms into 20 sub-tiles of
    128; sub-tile j has a fixed selection matrix S_j[128, 128] with
    S_j[p, m] = 1 iff (j*128+p)//20 == m.  These matrices are identical for
    every 2560-aligned atom group, so they're loaded to SBUF once.
  - float32r matmuls run at 1 cycle/row for free-dim >= 256; with 0/1 weights
    the products are exact, PSUM accumulates in fp32 -> exact fp32 sums.
  - Main loop: super-tiles of 4 groups (512 mols, 5.24 MB): 20 matmuls of
    free-dim 512 accumulate PSUM [128, 512]; ScalarE evicts with *1/20.
  - Tail per core: one 128-mol group (FD=128) + one 84-mol group (13 full
    sub-tiles + one 16-atom partial, handled by slicing S_13's partitions).
"""

import numpy as np

N_CORES = 8
TOTAL_ATOMS = 2_000_000
HIDDEN = 128
N_MOLS = 100_000
K = 20  # atoms per molecule
MOLS_PER_CORE = N_MOLS // N_CORES  # 12_500
ATOMS_PER_CORE = TOTAL_ATOMS // N_CORES  # 250_000

G = 4  # groups per super-tile
MOLS_PER_GROUP = 128
ATOMS_PER_GROUP = MOLS_PER_GROUP * K  # 2560
MOLS_PER_ST = G * MOLS_PER_GROUP  # 512
ATOMS_PER_ST = G * ATOMS_PER_GROUP  # 10240
N_ST = MOLS_PER_CORE // MOLS_PER_ST  # 24 full super-tiles
TAIL_MOLS = MOLS_PER_CORE - N_ST * MOLS_PER_ST  # 212
# tail tile A: 128 mols; tail tile B: 84 mols = 13 full 128-atom subtiles + 16
TAIL_A_MOLS = 128
TAIL_B_MOLS = TAIL_MOLS - TAIL_A_MOLS  # 84
TAIL_B_ATOMS = TAIL_B_MOLS * K  # 1680
TAIL_B_FULL_J = TAIL_B_ATOMS // 128  # 13
TAIL_B_REM = TAIL_B_ATOMS - TAIL_B_FULL_J * 128  # 16

_CACHE = {}


def _sel_matrices() -> np.ndarray:
    """sel[j, p, m] = 1.0 iff atom j*128+p belongs to molecule m (within a
    2560-atom group)."""
    j = np.arange(K)[:, None]
    p = np.arange(128)[None, :]
    mol = (j * 128 + p) // K  # [20, 128]
    sel = np.zeros((K, 128, MOLS_PER_GROUP), dtype=np.float32)
    jj, pp = np.nonzero(mol < MOLS_PER_GROUP)
    sel[jj, pp, mol[jj, pp]] = 1.0
    return sel


def _build_program():
    import concourse.bacc as bacc
    import concourse.tile as tile
    from concourse import mybir

    nc = bacc.Bacc("TRN2", target_bir_lowering=False, debug=False)

    f32 = mybir.dt.float32
    f32r = mybir.dt.float32r

    x = nc.dram_tensor("x", [ATOMS_PER_CORE, HIDDEN], f32r, kind="ExternalInput")
    sel = nc.dram_tensor("sel", [K, 128, MOLS_PER_GROUP], f32r, kind="ExternalInput")
    y = nc.dram_tensor("y", [MOLS_PER_CORE, HIDDEN], f32, kind="ExternalOutput")

    inv_k = 1.0 / K
    copy = mybir.ActivationFunctionType.Copy

    with tile.TileContext(nc) as tc:
        with (
            tc.tile_pool(name="selp", bufs=1) as selp,
            tc.tile_pool(name="inp", bufs=3) as inp,
            tc.tile_pool(name="outp", bufs=3) as outp,
            tc.tile_pool(name="psump", bufs=2, space="PSUM") as psump,
        ):
            sel_sb = selp.tile([128, K, MOLS_PER_GROUP], f32r)
            nc.sync.dma_start(out=sel_sb, in_=sel.rearrange("j p m -> p j m"))

            # ---- main super-tiles ----
            for s in range(N_ST):
                a0 = s * ATOMS_PER_ST
                in_t = inp.tile([128, K, G, HIDDEN], f32r, tag="in")
                for g in range(G):
                    ga = a0 + g * ATOMS_PER_GROUP
                    nc.sync.dma_start(
                        out=in_t[:, :, g, :],
                        in_=x[ga : ga + ATOMS_PER_GROUP, :].rearrange(
                            "(j p) h -> p j h", j=K, p=128
                        ),
                    )

                ps = psump.tile([128, G * HIDDEN], f32)
                for j in range(K):
                    nc.tensor.matmul(
                        ps,
                        lhsT=sel_sb[:, j, :],
                        rhs=in_t[:, j, :, :],
                        start=(j == 0),
                        stop=(j == K - 1),
                    )

                o_t = outp.tile([128, G, HIDDEN], f32, tag="out")
                nc.scalar.activation(o_t, ps, copy, scale=inv_k)
                dst = y[s * MOLS_PER_ST : (s + 1) * MOLS_PER_ST, :].rearrange(
                    "(g p) h -> p g h", g=G, p=128
                )
                nc.sync.dma_start(out=dst, in_=o_t)

            # ---- tail A: 128 mols, 2560 atoms ----
            a0 = N_ST * ATOMS_PER_ST
            m0 = N_ST * MOLS_PER_ST
            in_a = inp.tile([128, K, HIDDEN], f32r, tag="in")
            nc.sync.dma_start(
                out=in_a,
                in_=x[a0 : a0 + ATOMS_PER_GROUP, :].rearrange(
                    "(j p) h -> p j h", j=K, p=128
                ),
            )
            ps_a = psump.tile([128, HIDDEN], f32)
            for j in range(K):
                nc.tensor.matmul(
                    ps_a,
                    lhsT=sel_sb[:, j, :],
                    rhs=in_a[:, j, :],
                    start=(j == 0),
                    stop=(j == K - 1),
                )
            o_a = outp.tile([128, HIDDEN], f32, tag="out")
            nc.scalar.activation(o_a, ps_a, copy, scale=inv_k)
            nc.sync.dma_start(out=y[m0 : m0 + TAIL_A_MOLS, :], in_=o_a)

            # ---- tail B: 84 mols, 1680 atoms = 13x128 + 16 ----
            a0 += ATOMS_PER_GROUP
            m0 += TAIL_A_MOLS
            nj = TAIL_B_FULL_J + 1  # 14
            in_b = inp.tile([128, nj, HIDDEN], f32r, tag="in")
            nc.sync.dma_start(
                out=in_b[:, : TAIL_B_FULL_J, :],
                in_=x[a0 : a0 + TAIL_B_FULL_J * 128, :].rearrange(
                    "(j p) h -> p j h", j=TAIL_B_FULL_J, p=128
                ),
            )
            nc.sync.dma_start(
                out=in_b[:TAIL_B_REM, TAIL_B_FULL_J, :],
                in_=x[a0 + TAIL_B_FULL_J * 128 : a0 + TAIL_B_ATOMS, :],
            )
            ps_b = psump.tile([128, HIDDEN], f32)
            for j in range(TAIL_B_FULL_J):
                nc.tensor.matmul(
                    ps_b[:TAIL_B_MOLS, :],
                    lhsT=sel_sb[:, j, :TAIL_B_MOLS],
                    rhs=in_b[:, j, :],
                    start=(j == 0),
                    stop=False,
                )
            nc.tensor.matmul(
                ps_b[:TAIL_B_MOLS, :],
                lhsT=sel_sb[:TAIL_B_REM, TAIL_B_FULL_J, :TAIL_B_MOLS],
                rhs=in_b[:TAIL_B_REM, TAIL_B_FULL_J, :],
                start=False,
                stop=True,
            )
            o_b = outp.tile([128, HIDDEN], f32, tag="out")
            nc.scalar.activation(
                o_b[:TAIL_B_MOLS, :], ps_b[:TAIL_B_MOLS, :], copy, scale=inv_k
            )
            nc.sync.dma_start(out=y[m0 : m0 + TAIL_B_MOLS, :], in_=o_b[:TAIL_B_MOLS, :])

    nc.finalize()
    return nc


def _get_program():
    if "nc" not in _CACHE:
        _CACHE["nc"] = _build_program()
    return _CACHE["nc"]


def _uniform_pattern(segment_ids: np.ndarray, n_mols: int) -> bool:
    if segment_ids.shape != (TOTAL_ATOMS,) or n_mols != N_MOLS:
        return False
    expect = np.repeat(np.arange(N_MOLS, dtype=segment_ids.dtype), K)
    return bool(np.array_equal(segment_ids, expect))


def _numpy_fallback(atom_hiddens, segment_ids, n_mols):
    """Correct-but-slow path for non-uniform segment layouts (sorted ids)."""
    ah = np.asarray(atom_hiddens, dtype=np.float32)
    sid = np.asarray(segment_ids).astype(np.int64)
    counts = np.bincount(sid, minlength=n_mols).astype(np.float32)
    boundaries = np.searchsorted(sid, np.arange(n_mols))
    sums = np.add.reduceat(ah, boundaries, axis=0)
    # reduceat duplicates the next segment's sum for empty segments; zero them
    # (the reference module raises on empty molecules, so this is defensive).
    empty = counts == 0
    if empty.any():
        sums[empty] = 0.0
    return sums / np.maximum(counts, 1.0)[:, None]


def kernel(**inputs) -> np.ndarray:
    atom_hiddens = np.asarray(inputs["atom_hiddens"], dtype=np.float32)
    segment_ids = np.asarray(inputs["segment_ids"])
    n_mols = int(np.asarray(inputs["n_mols"]))

    if not _uniform_pattern(segment_ids, n_mols) or atom_hiddens.shape != (
        TOTAL_ATOMS,
        HIDDEN,
    ):
        return _numpy_fallback(atom_hiddens, segment_ids, n_mols)

    from concourse.bass_utils import run_bass_kernel_spmd

    nc = _get_program()
    sel = _sel_matrices()
    in_maps = [
        {
            "x": atom_hiddens[c * ATOMS_PER_CORE : (c + 1) * ATOMS_PER_CORE],
            "sel": sel,
        }
        for c in range(N_CORES)
    ]
    res = run_bass_kernel_spmd(nc, in_maps, core_ids=list(range(N_CORES)))
    return np.concatenate([r["y"] for r in res.results], axis=0)


if __name__ == "__main__":
    rng = np.random.default_rng(0)
    ah = rng.standard_normal((TOTAL_ATOMS, HIDDEN), dtype=np.float32)
    sid = np.repeat(np.arange(N_MOLS, dtype=np.int32), K)
    out = kernel(atom_hiddens=ah, segment_ids=sid, n_mols=N_MOLS)
    ref = ah.reshape(N_MOLS, K, HIDDEN).mean(axis=1)
    err = np.abs(out - ref).max() / max(np.abs(ref).max(), 1e-9)
    print("rel err:", err)


# revision 6
# speedup vs baseline: 1.0023x; 1.0023x over previous
"""Segment-mean (MeanAggregator) Trainium2 kernel.

Problem: atom_hiddens [2_000_000, 128] f32, segment_ids = repeat(arange(100_000), 20)
(uniform 20 atoms per molecule), output = per-molecule mean [100_000, 128] f32.

Strategy (8 NeuronCores, data-parallel over molecules):
  - Each core handles 12_500 molecules = 250_000 contiguous atom rows (128 MB).
  - Segment-sum is a matmul against constant 0/1 selection matrices:
      out[m, h] = sum_a S[a, m] * X[a, h]
    For a 2560-atom group (=128 molecules), split atoms into 20 sub-tiles of
    128; sub-tile j has a fixed selection matrix S_j[128, 128] with
    S_j[p, m] = 1 iff (j*128+p)//20 == m.  These matrices are identical for
    every 2560-aligned atom group, so they're loaded to SBUF once.
  - float32r matmuls run at 1 cycle/row for free-dim >= 256; with 0/1 weights
    the products are exact, PSUM accumulates in fp32 -> exact fp32 sums.
  - Main loop: super-tiles of 4 groups (512 mols, 5.24 MB): 20 matmuls of
    free-dim 512 accumulate PSUM [128, 512]; ScalarE evicts with *1/20.
  - Tail per core: one 128-mol group (FD=128) + one 84-mol group (13 full
    sub-tiles + one 16-atom partial, handled by slicing S_13's partitions).
"""

import numpy as np

N_CORES = 8
TOTAL_ATOMS = 2_000_000
HIDDEN = 128
N_MOLS = 100_000
K = 20  # atoms per molecule
MOLS_PER_CORE = N_MOLS // N_CORES  # 12_500
ATOMS_PER_CORE = TOTAL_ATOMS // N_CORES  # 250_000

G = 4  # groups per super-tile
MOLS_PER_GROUP = 128
ATOMS_PER_GROUP = MOLS_PER_GROUP * K  # 2560
MOLS_PER_ST = G * MOLS_PER_GROUP  # 512
ATOMS_PER_ST = G * ATOMS_PER_GROUP  # 10240
N_ST = MOLS_PER_CORE // MOLS_PER_ST  # 24 full super-tiles
TAIL_MOLS = MOLS_PER_CORE - N_ST * MOLS_PER_ST  # 212
# tail tile A: 128 mols; tail tile B: 84 mols = 13 full 128-atom subtiles + 16
TAIL_A_MOLS = 128
TAIL_B_MOLS = TAIL_MOLS - TAIL_A_MOLS  # 84
TAIL_B_ATOMS = TAIL_B_MOLS * K  # 1680
TAIL_B_FULL_J = TAIL_B_ATOMS // 128  # 13
TAIL_B_REM = TAIL_B_ATOMS - TAIL_B_FULL_J * 128  # 16

_CACHE = {}

# "float32" = exact 4-cycle/row matmuls; "float32r" = 1 cycle/row but
# truncates the moving operand to ~13 mantissa bits (measured 1.2e-4 rel err).
MM_DTYPE = "float32"


def _sel_matrices() -> np.ndarray:
    """sel[j, p, m] = 1.0 iff atom j*128+p belongs to molecule m (within a
    2560-atom group)."""
    j = np.arange(K)[:, None]
    p = np.arange(128)[None, :]
    mol = (j * 128 + p) // K  # [20, 128]
    sel = np.zeros((K, 128, MOLS_PER_GROUP), dtype=np.float32)
    jj, pp = np.nonzero(mol < MOLS_PER_GROUP)
    sel[jj, pp, mol[jj, pp]] = 1.0
    return sel


def _build_program():
    import concourse.bacc as bacc
    import concourse.tile as tile
    from concourse import mybir

    nc = bacc.Bacc("TRN2", target_bir_lowering=False, debug=False)

    f32 = mybir.dt.float32
    MM_DT = getattr(mybir.dt, MM_DTYPE)

    x = nc.dram_tensor("x", [ATOMS_PER_CORE, HIDDEN], MM_DT, kind="ExternalInput")
    sel = nc.dram_tensor("sel", [K, 128, MOLS_PER_GROUP], MM_DT, kind="ExternalInput")
    y = nc.dram_tensor("y", [MOLS_PER_CORE, HIDDEN], f32, kind="ExternalOutput")

    inv_k = 1.0 / K
    copy = mybir.ActivationFunctionType.Copy

    with tile.TileContext(nc) as tc:
        with (
            tc.tile_pool(name="selp", bufs=1) as selp,
            tc.tile_pool(name="inp", bufs=3) as inp,
            tc.tile_pool(name="outp", bufs=3) as outp,
            tc.tile_pool(name="psump", bufs=2, space="PSUM") as psump,
        ):
            sel_sb = selp.tile([128, K, MOLS_PER_GROUP], MM_DT)
            nc.sync.dma_start(out=sel_sb, in_=sel.rearrange("j p m -> p j m"))

            # ---- main super-tiles ----
            for s in range(N_ST):
                a0 = s * ATOMS_PER_ST
                in_t = inp.tile([128, K, G, HIDDEN], MM_DT, tag="in")
                for g in range(G):
                    ga = a0 + g * ATOMS_PER_GROUP
                    nc.sync.dma_start(
                        out=in_t[:, :, g, :],
                        in_=x[ga : ga + ATOMS_PER_GROUP, :].rearrange(
                            "(j p) h -> p j h", j=K, p=128
                        ),
                    )

                ps = psump.tile([128, G * HIDDEN], f32)
                for j in range(K):
                    nc.tensor.matmul(
                        ps,
                        lhsT=sel_sb[:, j, :],
                        rhs=in_t[:, j, :, :],
                        start=(j == 0),
                        stop=(j == K - 1),
                    )

                o_t = outp.tile([128, G, HIDDEN], f32, tag="out")
                nc.scalar.activation(o_t, ps, copy, scale=inv_k)
                dst = y[s * MOLS_PER_ST : (s + 1) * MOLS_PER_ST, :].rearrange(
                    "(g p) h -> p g h", g=G, p=128
                )
                nc.sync.dma_start(out=dst, in_=o_t)

            # ---- tail A: 128 mols, 2560 atoms ----
            a0 = N_ST * ATOMS_PER_ST
            m0 = N_ST * MOLS_PER_ST
            in_a = inp.tile([128, K, HIDDEN], MM_DT, tag="in")
            nc.sync.dma_start(
                out=in_a,
                in_=x[a0 : a0 + ATOMS_PER_GROUP, :].rearrange(
                    "(j p) h -> p j h", j=K, p=128
                ),
            )
            ps_a = psump.tile([128, HIDDEN], f32)
            for j in range(K):
                nc.tensor.matmul(
                    ps_a,
                    lhsT=sel_sb[:, j, :],
                    rhs=in_a[:, j, :],
                    start=(j == 0),
                    stop=(j == K - 1),
                )
            o_a = outp.tile([128, HIDDEN], f32, tag="out")
            nc.scalar.activation(o_a, ps_a, copy, scale=inv_k)
            nc.sync.dma_start(out=y[m0 : m0 + TAIL_A_MOLS, :], in_=o_a)

            # ---- tail B: 84 mols, 1680 atoms = 13x128 + 16 ----
            a0 += ATOMS_PER_GROUP
            m0 += TAIL_A_MOLS
            nj = TAIL_B_FULL_J + 1  # 14
            in_b = inp.tile([128, nj, HIDDEN], MM_DT, tag="in")
            nc.sync.dma_start(
                out=in_b[:, : TAIL_B_FULL_J, :],
                in_=x[a0 : a0 + TAIL_B_FULL_J * 128, :].rearrange(
                    "(j p) h -> p j h", j=TAIL_B_FULL_J, p=128
                ),
            )
            nc.sync.dma_start(
                out=in_b[:TAIL_B_REM, TAIL_B_FULL_J, :],
                in_=x[a0 + TAIL_B_FULL_J * 128 : a0 + TAIL_B_ATOMS, :],
            )
            ps_b = psump.tile([128, HIDDEN], f32)
            for j in range(TAIL_B_FULL_J):
                nc.tensor.matmul(
                    ps_b[:TAIL_B_MOLS, :],
                    lhsT=sel_sb[:, j, :TAIL_B_MOLS],
                    rhs=in_b[:, j, :],
                    start=(j == 0),
                    stop=False,
                )
            nc.tensor.matmul(
                ps_b[:TAIL_B_MOLS, :],
                lhsT=sel_sb[:TAIL_B_REM, TAIL_B_FULL_J, :TAIL_B_MOLS],
                rhs=in_b[:TAIL_B_REM, TAIL_B_FULL_J, :],
                start=False,
                stop=True,
            )
            o_b = outp.tile([128, HIDDEN], f32, tag="out")
            nc.scalar.activation(
                o_b[:TAIL_B_MOLS, :], ps_b[:TAIL_B_MOLS, :], copy, scale=inv_k
            )
            nc.sync.dma_start(out=y[m0 : m0 + TAIL_B_MOLS, :], in_=o_b[:TAIL_B_MOLS, :])

    nc.finalize()
    return nc


def _get_program():
    if "nc" not in _CACHE:
        _CACHE["nc"] = _build_program()
    return _CACHE["nc"]


def _uniform_pattern(segment_ids: np.ndarray, n_mols: int) -> bool:
    if segment_ids.shape != (TOTAL_ATOMS,) or n_mols != N_MOLS:
        return False
    expect = np.repeat(np.arange(N_MOLS, dtype=segment_ids.dtype), K)
    return bool(np.array_equal(segment_ids, expect))


def _numpy_fallback(atom_hiddens, segment_ids, n_mols):
    """Correct-but-slow path for non-uniform segment layouts (sorted ids)."""
    ah = np.asarray(atom_hiddens, dtype=np.float32)
    sid = np.asarray(segment_ids).astype(np.int64)
    counts = np.bincount(sid, minlength=n_mols).astype(np.float32)
    boundaries = np.searchsorted(sid, np.arange(n_mols))
    sums = np.add.reduceat(ah, boundaries, axis=0)
    # reduceat duplicates the next segment's sum for empty segments; zero them
    # (the reference module raises on empty molecules, so this is defensive).
    empty = counts == 0
    if empty.any():
        sums[empty] = 0.0
    return sums / np.maximum(counts, 1.0)[:, None]


def kernel(**inputs) -> np.ndarray:
    atom_hiddens = np.asarray(inputs["atom_hiddens"], dtype=np.float32)
    segment_ids = np.asarray(inputs["segment_ids"])
    n_mols = int(np.asarray(inputs["n_mols"]))

    if not _uniform_pattern(segment_ids, n_mols) or atom_hiddens.shape != (
        TOTAL_ATOMS,
        HIDDEN,
    ):
        return _numpy_fallback(atom_hiddens, segment_ids, n_mols)

    from concourse.bass_utils import run_bass_kernel_spmd

    nc = _get_program()
    sel = _sel_matrices()
    in_maps = [
        {
            "x": atom_hiddens[c * ATOMS_PER_CORE : (c + 1) * ATOMS_PER_CORE],
            "sel": sel,
        }
        for c in range(N_CORES)
    ]
    res = run_bass_kernel_spmd(nc, in_maps, core_ids=list(range(N_CORES)))
    return np.concatenate([r["y"] for r in res.results], axis=0)


if __name__ == "__main__":
    rng = np.random.default_rng(0)
    ah = rng.standard_normal((TOTAL_ATOMS, HIDDEN), dtype=np.float32)
    sid = np.repeat(np.arange(N_MOLS, dtype=np.int32), K)
    out = kernel(atom_hiddens=ah, segment_ids=sid, n_mols=N_MOLS)
    ref = ah.reshape(N_MOLS, K, HIDDEN).mean(axis=1)
    err = np.abs(out - ref).max() / max(np.abs(ref).max(), 1e-9)
    print("rel err:", err)


# revision 8
# speedup vs baseline: 1.1715x; 1.1688x over previous
"""Segment-mean (MeanAggregator) Trainium2 kernel.

Problem: atom_hiddens [2_000_000, 128] f32, segment_ids = repeat(arange(100_000), 20)
(uniform 20 atoms per molecule), output = per-molecule mean [100_000, 128] f32.

Strategy (8 NeuronCores, data-parallel over molecules):
  - Each core handles 12_500 molecules = 250_000 contiguous atom rows (128 MB).
  - Molecule-per-partition layout: partition p of a group holds molecule
    g*128+p as 20 contiguous rows (r, h) = 2560 f32 = 10 KB.  One DMA per
    super-tile (4 groups, 5.24 MB) with fully-contiguous 10 KB per-partition
    runs -> near-peak HBM bandwidth (512 B-chunk layouts measured ~290 GB/s;
    contiguous runs ~355 GB/s).
  - The 20-chunk reduction sum_r tile[p, g, r, :] is split between two engines
    so neither is the bottleneck:
      * PE groups: 20 accumulating fp32 matmuls with an IDENTITY stationary
        operand (partition-preserving accumulate into PSUM).  fp32 matmul is
        4 cycles/row (exact); FD=256 (2 groups) per instruction.
      * DVE groups: tensor_reduce over a permuted AP [p, h, r] (axis=X).
    Both are exact fp32 adds.  ScalarE evicts/scales by 1/20; output DMAs go
    out on the second HWDGE ring (nc.scalar) to keep the SP ring input-only.
  - Tail per core: 212 mols = one 128-mol group + one 84-mol group, both via
    identity matmuls FD=128.
"""

import numpy as np

N_CORES = 8
TOTAL_ATOMS = 2_000_000
HIDDEN = 128
N_MOLS = 100_000
K = 20  # atoms per molecule
MOLS_PER_CORE = N_MOLS // N_CORES  # 12_500
ATOMS_PER_CORE = TOTAL_ATOMS // N_CORES  # 250_000

G = 4  # groups per super-tile
PE_G = 2  # groups 0..PE_G-1 reduced on PE; the rest on DVE
MOLS_PER_GROUP = 128
ATOMS_PER_GROUP = MOLS_PER_GROUP * K  # 2560
MOLS_PER_ST = G * MOLS_PER_GROUP  # 512
ATOMS_PER_ST = G * ATOMS_PER_GROUP  # 10240
N_ST = MOLS_PER_CORE // MOLS_PER_ST  # 24 full super-tiles
TAIL_MOLS = MOLS_PER_CORE - N_ST * MOLS_PER_ST  # 212
TAIL_A_MOLS = 128
TAIL_B_MOLS = TAIL_MOLS - TAIL_A_MOLS  # 84

_CACHE = {}


def _build_program():
    import concourse.bacc as bacc
    import concourse.tile as tile
    from concourse import mybir

    nc = bacc.Bacc("TRN2", target_bir_lowering=False, debug=False)

    f32 = mybir.dt.float32

    x = nc.dram_tensor("x", [ATOMS_PER_CORE, HIDDEN], f32, kind="ExternalInput")
    ident = nc.dram_tensor("ident", [128, 128], f32, kind="ExternalInput")
    y = nc.dram_tensor("y", [MOLS_PER_CORE, HIDDEN], f32, kind="ExternalOutput")

    inv_k = 1.0 / K
    copy = mybir.ActivationFunctionType.Copy
    AX = mybir.AxisListType.X

    with tile.TileContext(nc) as tc:
        with (
            tc.tile_pool(name="constp", bufs=1) as constp,
            tc.tile_pool(name="inp", bufs=3) as inp,
            tc.tile_pool(name="redp", bufs=4) as redp,
            tc.tile_pool(name="outp", bufs=3) as outp,
            tc.tile_pool(name="psump", bufs=2, space="PSUM") as psump,
        ):
            ident_sb = constp.tile([128, 128], f32)
            nc.sync.dma_start(out=ident_sb, in_=ident[:, :])

            def reduce_group_dve(in_t, g, o_t):
                red = redp.tile([128, HIDDEN], f32, tag="red")
                nc.vector.reduce_sum(
                    out=red, in_=in_t[:, g, :, :].rearrange("p r h -> p h r"), axis=AX
                )
                nc.scalar.activation(o_t[:, g, :], red, copy, scale=inv_k)

            # ---- main super-tiles ----
            for s in range(N_ST):
                a0 = s * ATOMS_PER_ST
                in_t = inp.tile([128, G, K, HIDDEN], f32, tag="in")
                nc.sync.dma_start(
                    out=in_t,
                    in_=x[a0 : a0 + ATOMS_PER_ST, :].rearrange(
                        "(g p r) h -> p g r h", g=G, p=128, r=K
                    ),
                )

                o_t = outp.tile([128, G, HIDDEN], f32, tag="out")

                ps = psump.tile([128, PE_G * HIDDEN], f32)
                for r in range(K):
                    nc.tensor.matmul(
                        ps,
                        lhsT=ident_sb,
                        rhs=in_t[:, 0:PE_G, r, :],
                        start=(r == 0),
                        stop=(r == K - 1),
                    )
                nc.scalar.activation(o_t[:, 0:PE_G, :], ps, copy, scale=inv_k)

                for g in range(PE_G, G):
                    reduce_group_dve(in_t, g, o_t)

                dst = y[s * MOLS_PER_ST : (s + 1) * MOLS_PER_ST, :].rearrange(
                    "(g p) h -> p g h", g=G, p=128
                )
                nc.scalar.dma_start(out=dst, in_=o_t)

            # ---- tail A: 128 mols ----
            a0 = N_ST * ATOMS_PER_ST
            m0 = N_ST * MOLS_PER_ST
            in_a = inp.tile([128, K, HIDDEN], f32, tag="in")
            nc.sync.dma_start(
                out=in_a,
                in_=x[a0 : a0 + ATOMS_PER_GROUP, :].rearrange(
                    "(p r) h -> p r h", p=128, r=K
                ),
            )
            ps_a = psump.tile([128, HIDDEN], f32)
            for r in range(K):
                nc.tensor.matmul(
                    ps_a,
                    lhsT=ident_sb,
                    rhs=in_a[:, r, :],
                    start=(r == 0),
                    stop=(r == K - 1),
                )
            o_a = outp.tile([128, HIDDEN], f32, tag="out")
            nc.scalar.activation(o_a, ps_a, copy, scale=inv_k)
            nc.scalar.dma_start(out=y[m0 : m0 + TAIL_A_MOLS, :], in_=o_a)

            # ---- tail B: 84 mols ----
            a0 += ATOMS_PER_GROUP
            m0 += TAIL_A_MOLS
            in_b = inp.tile([128, K, HIDDEN], f32, tag="in")
            nc.sync.dma_start(
                out=in_b[:TAIL_B_MOLS],
                in_=x[a0 : a0 + TAIL_B_MOLS * K, :].rearrange(
                    "(p r) h -> p r h", p=TAIL_B_MOLS, r=K
                ),
            )
            ps_b = psump.tile([128, HIDDEN], f32)
            for r in range(K):
                nc.tensor.matmul(
                    ps_b[:TAIL_B_MOLS, :],
                    lhsT=ident_sb[:TAIL_B_MOLS, :TAIL_B_MOLS],
                    rhs=in_b[:TAIL_B_MOLS, r, :],
                    start=(r == 0),
                    stop=(r == K - 1),
                )
            o_b = outp.tile([128, HIDDEN], f32, tag="out")
            nc.scalar.activation(
                o_b[:TAIL_B_MOLS, :], ps_b[:TAIL_B_MOLS, :], copy, scale=inv_k
            )
            nc.scalar.dma_start(
                out=y[m0 : m0 + TAIL_B_MOLS, :], in_=o_b[:TAIL_B_MOLS, :]
            )

    nc.finalize()
    return nc


def _get_program():
    if "nc" not in _CACHE:
        _CACHE["nc"] = _build_program()
    return _CACHE["nc"]


def _uniform_pattern(segment_ids: np.ndarray, n_mols: int) -> bool:
    if segment_ids.shape != (TOTAL_ATOMS,) or n_mols != N_MOLS:
        return False
    expect = np.repeat(np.arange(N_MOLS, dtype=segment_ids.dtype), K)
    return bool(np.array_equal(segment_ids, expect))


def _numpy_fallback(atom_hiddens, segment_ids, n_mols):
    """Correct-but-slow path for non-uniform segment layouts (sorted ids)."""
    ah = np.asarray(atom_hiddens, dtype=np.float32)
    sid = np.asarray(segment_ids).astype(np.int64)
    counts = np.bincount(sid, minlength=n_mols).astype(np.float32)
    boundaries = np.searchsorted(sid, np.arange(n_mols))
    sums = np.add.reduceat(ah, boundaries, axis=0)
    empty = counts == 0
    if empty.any():
        sums[empty] = 0.0
    return sums / np.maximum(counts, 1.0)[:, None]


def kernel(**inputs) -> np.ndarray:
    atom_hiddens = np.asarray(inputs["atom_hiddens"], dtype=np.float32)
    segment_ids = np.asarray(inputs["segment_ids"])
    n_mols = int(np.asarray(inputs["n_mols"]))

    if not _uniform_pattern(segment_ids, n_mols) or atom_hiddens.shape != (
        TOTAL_ATOMS,
        HIDDEN,
    ):
        return _numpy_fallback(atom_hiddens, segment_ids, n_mols)

    from concourse.bass_utils import run_bass_kernel_spmd

    nc = _get_program()
    ident = np.eye(128, dtype=np.float32)
    in_maps = [
        {
            "x": atom_hiddens[c * ATOMS_PER_CORE : (c + 1) * ATOMS_PER_CORE],
            "ident": ident,
        }
        for c in range(N_CORES)
    ]
    res = run_bass_kernel_spmd(nc, in_maps, core_ids=list(range(N_CORES)))
    return np.concatenate([r["y"] for r in res.results], axis=0)


if __name__ == "__main__":
    rng = np.random.default_rng(0)
    ah = rng.standard_normal((TOTAL_ATOMS, HIDDEN), dtype=np.float32)
    sid = np.repeat(np.arange(N_MOLS, dtype=np.int32), K)
    out = kernel(atom_hiddens=ah, segment_ids=sid, n_mols=N_MOLS)
    ref = ah.reshape(N_MOLS, K, HIDDEN).mean(axis=1)
    err = np.abs(out - ref).max() / max(np.abs(ref).max(), 1e-9)
    print("rel err:", err)


# revision 11
# speedup vs baseline: 1.2361x; 1.0551x over previous
"""Segment-mean (MeanAggregator) Trainium2 kernel.

Problem: atom_hiddens [2_000_000, 128] f32, segment_ids = repeat(arange(100_000), 20)
(uniform 20 atoms per molecule), output = per-molecule mean [100_000, 128] f32.

Strategy (8 NeuronCores, data-parallel over molecules):
  - Each core handles 12_500 molecules = 250_000 contiguous atom rows (128 MB).
  - Molecule-per-partition layout: partition p of a group holds molecule
    g*128+p as 20 contiguous rows (r, h) = 2560 f32 = 10 KB.  One DMA per
    super-tile (4 groups, 5.24 MB) with fully-contiguous 10 KB per-partition
    runs -> near-peak HBM bandwidth (512 B-chunk layouts measured ~290 GB/s;
    contiguous runs ~355 GB/s).
  - The 20-chunk reduction sum_r tile[p, g, r, :] is split between two engines
    so neither is the bottleneck:
      * PE groups: 20 accumulating fp32 matmuls with an IDENTITY stationary
        operand (partition-preserving accumulate into PSUM).  fp32 matmul is
        4 cycles/row (exact); FD=256 (2 groups) per instruction.
      * DVE groups: tensor_reduce over a permuted AP [p, h, r] (axis=X).
    Both are exact fp32 adds.  ScalarE evicts/scales by 1/20; output DMAs go
    out on the second HWDGE ring (nc.scalar) to keep the SP ring input-only.
  - Tail per core: 212 mols = one 128-mol group + one 84-mol group, both via
    identity matmuls FD=128.
"""

import numpy as np

N_CORES = 8
TOTAL_ATOMS = 2_000_000
HIDDEN = 128
N_MOLS = 100_000
K = 20  # atoms per molecule
MOLS_PER_CORE = N_MOLS // N_CORES  # 12_500
ATOMS_PER_CORE = TOTAL_ATOMS // N_CORES  # 250_000

G = 4  # groups per super-tile
PE_G = 1  # groups 0..PE_G-1 reduced on PE; the rest on DVE (one fused reduce)
MOLS_PER_GROUP = 128
ATOMS_PER_GROUP = MOLS_PER_GROUP * K  # 2560
MOLS_PER_ST = G * MOLS_PER_GROUP  # 512
ATOMS_PER_ST = G * ATOMS_PER_GROUP  # 10240
N_ST = MOLS_PER_CORE // MOLS_PER_ST  # 24 full super-tiles
TAIL_MOLS = MOLS_PER_CORE - N_ST * MOLS_PER_ST  # 212
TAIL_A_MOLS = 128
TAIL_B_MOLS = TAIL_MOLS - TAIL_A_MOLS  # 84

_CACHE = {}


def _build_program():
    import concourse.bacc as bacc
    import concourse.tile as tile
    from concourse import mybir

    nc = bacc.Bacc("TRN2", target_bir_lowering=False, debug=False)

    f32 = mybir.dt.float32

    x = nc.dram_tensor("x", [ATOMS_PER_CORE, HIDDEN], f32, kind="ExternalInput")
    ident = nc.dram_tensor("ident", [128, 128], f32, kind="ExternalInput")
    y = nc.dram_tensor("y", [MOLS_PER_CORE, HIDDEN], f32, kind="ExternalOutput")

    inv_k = 1.0 / K
    copy = mybir.ActivationFunctionType.Copy
    AX = mybir.AxisListType.X

    with tile.TileContext(nc) as tc:
        with (
            tc.tile_pool(name="constp", bufs=1) as constp,
            tc.tile_pool(name="inp", bufs=3) as inp,
            tc.tile_pool(name="redp", bufs=4) as redp,
            tc.tile_pool(name="outp", bufs=3) as outp,
            tc.tile_pool(name="psump", bufs=2, space="PSUM") as psump,
        ):
            ident_sb = constp.tile([128, 128], f32)
            nc.sync.dma_start(out=ident_sb, in_=ident[:, :])

            # ---- main super-tiles ----
            for s in range(N_ST):
                a0 = s * ATOMS_PER_ST
                in_t = inp.tile([128, G, K, HIDDEN], f32, tag="in")
                nc.sync.dma_start(
                    out=in_t,
                    in_=x[a0 : a0 + ATOMS_PER_ST, :].rearrange(
                        "(g p r) h -> p g r h", g=G, p=128, r=K
                    ),
                )

                o_t = outp.tile([128, G, HIDDEN], f32, tag="out")

                ps = psump.tile([128, PE_G * HIDDEN], f32)
                for r in range(K):
                    nc.tensor.matmul(
                        ps,
                        lhsT=ident_sb,
                        rhs=in_t[:, 0:PE_G, r, :],
                        start=(r == 0),
                        stop=(r == K - 1),
                    )
                nc.scalar.activation(o_t[:, 0:PE_G, :], ps, copy, scale=inv_k)

                # one fused DVE reduce for groups PE_G..G-1: innermost axis r
                red = redp.tile([128, G - PE_G, HIDDEN], f32, tag="red")
                nc.vector.reduce_sum(
                    out=red,
                    in_=in_t[:, PE_G:G, :, :].rearrange("p g r h -> p g h r"),
                    axis=AX,
                )
                nc.scalar.activation(o_t[:, PE_G:G, :], red, copy, scale=inv_k)

                dst = y[s * MOLS_PER_ST : (s + 1) * MOLS_PER_ST, :].rearrange(
                    "(g p) h -> p g h", g=G, p=128
                )
                nc.scalar.dma_start(out=dst, in_=o_t)

            # ---- tail A: 128 mols ----
            a0 = N_ST * ATOMS_PER_ST
            m0 = N_ST * MOLS_PER_ST
            in_a = inp.tile([128, K, HIDDEN], f32, tag="in")
            nc.sync.dma_start(
                out=in_a,
                in_=x[a0 : a0 + ATOMS_PER_GROUP, :].rearrange(
                    "(p r) h -> p r h", p=128, r=K
                ),
            )
            ps_a = psump.tile([128, HIDDEN], f32)
            for r in range(K):
                nc.tensor.matmul(
                    ps_a,
                    lhsT=ident_sb,
                    rhs=in_a[:, r, :],
                    start=(r == 0),
                    stop=(r == K - 1),
                )
            o_a = outp.tile([128, HIDDEN], f32, tag="out")
            nc.scalar.activation(o_a, ps_a, copy, scale=inv_k)
            nc.scalar.dma_start(out=y[m0 : m0 + TAIL_A_MOLS, :], in_=o_a)

            # ---- tail B: 84 mols ----
            a0 += ATOMS_PER_GROUP
            m0 += TAIL_A_MOLS
            in_b = inp.tile([128, K, HIDDEN], f32, tag="in")
            nc.sync.dma_start(
                out=in_b[:TAIL_B_MOLS],
                in_=x[a0 : a0 + TAIL_B_MOLS * K, :].rearrange(
                    "(p r) h -> p r h", p=TAIL_B_MOLS, r=K
                ),
            )
            ps_b = psump.tile([128, HIDDEN], f32)
            for r in range(K):
                nc.tensor.matmul(
                    ps_b[:TAIL_B_MOLS, :],
                    lhsT=ident_sb[:TAIL_B_MOLS, :TAIL_B_MOLS],
                    rhs=in_b[:TAIL_B_MOLS, r, :],
                    start=(r == 0),
                    stop=(r == K - 1),
                )
            o_b = outp.tile([128, HIDDEN], f32, tag="out")
            nc.scalar.activation(
                o_b[:TAIL_B_MOLS, :], ps_b[:TAIL_B_MOLS, :], copy, scale=inv_k
            )
            nc.scalar.dma_start(
                out=y[m0 : m0 + TAIL_B_MOLS, :], in_=o_b[:TAIL_B_MOLS, :]
            )

    nc.finalize()
    return nc


def _get_program():
    if "nc" not in _CACHE:
        _CACHE["nc"] = _build_program()
    return _CACHE["nc"]


def _uniform_pattern(segment_ids: np.ndarray, n_mols: int) -> bool:
    if segment_ids.shape != (TOTAL_ATOMS,) or n_mols != N_MOLS:
        return False
    expect = np.repeat(np.arange(N_MOLS, dtype=segment_ids.dtype), K)
    return bool(np.array_equal(segment_ids, expect))


def _numpy_fallback(atom_hiddens, segment_ids, n_mols):
    """Correct-but-slow path for non-uniform segment layouts (sorted ids)."""
    ah = np.asarray(atom_hiddens, dtype=np.float32)
    sid = np.asarray(segment_ids).astype(np.int64)
    counts = np.bincount(sid, minlength=n_mols).astype(np.float32)
    boundaries = np.searchsorted(sid, np.arange(n_mols))
    sums = np.add.reduceat(ah, boundaries, axis=0)
    empty = counts == 0
    if empty.any():
        sums[empty] = 0.0
    return sums / np.maximum(counts, 1.0)[:, None]


def kernel(**inputs) -> np.ndarray:
    atom_hiddens = np.asarray(inputs["atom_hiddens"], dtype=np.float32)
    segment_ids = np.asarray(inputs["segment_ids"])
    n_mols = int(np.asarray(inputs["n_mols"]))

    if not _uniform_pattern(segment_ids, n_mols) or atom_hiddens.shape != (
        TOTAL_ATOMS,
        HIDDEN,
    ):
        return _numpy_fallback(atom_hiddens, segment_ids, n_mols)

    from concourse.bass_utils import run_bass_kernel_spmd

    nc = _get_program()
    ident = np.eye(128, dtype=np.float32)
    in_maps = [
        {
            "x": atom_hiddens[c * ATOMS_PER_CORE : (c + 1) * ATOMS_PER_CORE],
            "ident": ident,
        }
        for c in range(N_CORES)
    ]
    res = run_bass_kernel_spmd(nc, in_maps, core_ids=list(range(N_CORES)))
    return np.concatenate([r["y"] for r in res.results], axis=0)


if __name__ == "__main__":
    rng = np.random.default_rng(0)
    ah = rng.standard_normal((TOTAL_ATOMS, HIDDEN), dtype=np.float32)
    sid = np.repeat(np.arange(N_MOLS, dtype=np.int32), K)
    out = kernel(atom_hiddens=ah, segment_ids=sid, n_mols=N_MOLS)
    ref = ah.reshape(N_MOLS, K, HIDDEN).mean(axis=1)
    err = np.abs(out - ref).max() / max(np.abs(ref).max(), 1e-9)
    print("rel err:", err)
